# revision 21
# baseline (speedup 1.0000x reference)
"""Causal self-attention on 8 Trainium2 NeuronCores (Bass/Tile).

Problem: y = CausalSelfAttention(x; Wq, Wk, Wv, Wo) with
B=4, S=2048, E=1024, H=16 heads of 64, fp32 inputs/outputs.

Sharding (tensor-parallel x data-parallel): core c of 8 handles batch
b = c//2 and head-group g = c%2 (8 of 16 heads). Each core receives
x[b] [S, E], the head-group's columns of Wq/Wk/Wv [E, 512] and rows of
Wo [512, E], and produces a partial output projection [S, E]. The host
sums the two partials per batch.

Per-core dataflow (attention-path operands bf16, accumulation fp32),
restructured from the v1 kernel for engine balance:

  - ST [128 k, 2x512 q] = K @ Q.T per head-pair per k-tile (K=64 halves).
  - EXP on ACT engine (one wide ACTIVATE per k-tile; causal mask via
    triangular multiplicative mask on the diagonal subblock, on DVE).
  - PV repacked [q, d]-major: lhsT = pt chunk [128 k, 128 q],
    rhs = [V_h | 1] [128 k, 65] -> av [128 q, 65] PSUM accumulated over
    k-tiles at full 128-wide array utilization; the ones column gives
    per-partition softmax sums. PSUM accumulation-group regions within
    a bank must be SERIALIZED (interleaved per-region groups corrupt on
    HW - measured), so PV runs qt-outer over persistent pt tiles, and
    the whole PV phase of chunk qc is software-pipelined into the
    ST/EXP stream of chunk qc+1.
  - normalize = per-partition tensor_scalar (DVE); 1/sums via DVE
    reciprocal_approx_fast; causal mask multiply on GpSimd.
  - attT via XBAR DMA transpose (SBUF->SBUF) for pairs 0-2 (latency
    slack), PE transpose + ACT evac for pair 3 (feeds outproj).
  - output projection PSUM-accumulated over the 4 head-pairs per
    q-tile (no SBUF accumulation), DMA'd out from an SBUF bounce.
  - QKV projections software-pipelined INTO the attention stream (a
    work queue pumped once per k-tile iteration) so there is no serial
    projection phase.
"""

import collections

import numpy as np

import concourse.bass as bass
import concourse.mybir as mybir
from concourse import bacc
from concourse.masks import make_identity
from concourse.tile import TileContext

FP = mybir.dt.float32
BF = mybir.dt.bfloat16
P = 128


def build(S=2048, E=1024, HPC=8, DH=64, NQ=512):
    GD = HPC * DH        # 512 head dims per core
    KT_E = E // P        # 8 contraction tiles over E
    ST_S = S // P        # 16 seq tiles (also key tiles)
    QC = S // NQ         # 4 query chunks of 512
    PAIRS = HPC // 2     # 4 head pairs
    QSUB = NQ // P       # 4 q-tiles of 128 per chunk
    VW = DH + 1          # 65: V columns + ones column

    assert DH == 64 and NQ % P == 0 and S % NQ == 0 and E % P == 0

    nc = bacc.Bacc(None, target_bir_lowering=False)
    x_d = nc.dram_tensor("x", [S, E], BF, kind="ExternalInput")
    wq_d = nc.dram_tensor("wq", [E, GD], BF, kind="ExternalInput")
    wk_d = nc.dram_tensor("wk", [E, GD], BF, kind="ExternalInput")
    wv_d = nc.dram_tensor("wv", [E, GD], BF, kind="ExternalInput")
    wo_d = nc.dram_tensor("wo", [GD, E], BF, kind="ExternalInput")
    out_d = nc.dram_tensor("out", [S, E], FP, kind="ExternalOutput")

    with TileContext(nc) as tc:
        with (
            tc.tile_pool(name="consts", bufs=1) as consts,
            tc.tile_pool(name="xt", bufs=1) as xt_pool,
            tc.tile_pool(name="wbuf", bufs=1) as wbuf,
            tc.tile_pool(name="qkv", bufs=1) as qkv,
            tc.tile_pool(name="pt", bufs=22) as pt_pool,
            tc.tile_pool(name="att2", bufs=3) as att2_pool,
            tc.tile_pool(name="attTp", bufs=1) as attT_pool,
            tc.tile_pool(name="rcp", bufs=4) as rcp_pool,
            tc.tile_pool(name="outsb", bufs=3) as out_sb_pool,
            tc.tile_pool(name="st_ps", bufs=2, space="PSUM") as st_psum,
            tc.tile_pool(name="av_ps", bufs=1, space="PSUM") as av_psum,
            tc.tile_pool(name="mm_ps", bufs=2, space="PSUM") as mm_psum,
        ):
            # ---- constants -------------------------------------------------
            ones_bf = consts.tile([P, HPC], BF)
            nc.vector.memset(ones_bf[:], 1.0)
            identity = consts.tile([P, P], BF)
            make_identity(nc, identity)
            # upper-triangular-inclusive multiplicative mask (valid k <= q)
            ut_mask = consts.tile([P, P], BF)
            nc.gpsimd.memset(ut_mask[:], 0.0)
            nc.gpsimd.affine_select(
                out=ut_mask[:], in_=ut_mask[:],
                compare_op=mybir.AluOpType.is_gt, fill=1.0,
                base=0, pattern=[[-1, P]], channel_multiplier=1,
            )

            # ---- persistent SBUF tensors ----------------------------------
            xt_c = [
                [xt_pool.tile([P, NQ], BF, tag=f"xt{et}_{sc}", name=f"xt{et}_{sc}")
                 for sc in range(QC)]
                for et in range(KT_E)
            ]

            def xt(kt, lo, width):
                sc, r = divmod(lo, NQ)
                assert r + width <= NQ
                return xt_c[kt][sc][:, r : r + width]

            wv_sb = [wbuf.tile([P, GD], BF, tag=f"wv{kt}", name=f"wv{kt}")
                     for kt in range(KT_E)]
            wo_sb = [wbuf.tile([P, E], BF, tag=f"wo{p}", name=f"wo{p}")
                     for p in range(PAIRS)]
            # all wq/wk tiles persistent (16 KB/partition total)
            wkq = [
                [
                    [wbuf.tile([P, P], BF, tag=f"w{p}_{pr}_{kt}",
                               name=f"w{p}_{pr}_{kt}") for kt in range(KT_E)]
                    for pr in range(2)  # 0 = k, 1 = q
                ]
                for p in range(PAIRS)
            ]

            kT = [qkv.tile([P, S], BF, tag=f"kT{p}", name=f"kT{p}")
                  for p in range(PAIRS)]
            qT = [qkv.tile([P, S], BF, tag=f"qT{p}", name=f"qT{p}")
                  for p in range(PAIRS)]
            v = [qkv.tile([P, HPC * VW], BF, tag=f"v{st}", name=f"v{st}")
                 for st in range(ST_S)]

            attT = [
                [attT_pool.tile([P, P], BF, tag=f"attT{p}_{g}",
                                name=f"attT{p}_{g}") for g in range(ST_S)]
                for p in range(PAIRS)
            ]

            # ---- input DMAs ------------------------------------------------
            for kt in range(KT_E):
                nc.sync.dma_start(out=wv_sb[kt][:],
                                  in_=wv_d[kt * P : (kt + 1) * P, :])
            # x transposes, first seq chunk first so v/kq start early.
            # Chunk 0 is split sync/scalar (ACT has no work yet, and its 4
            # transposes finish before its first v-evac) to halve the time
            # to first matmul; later chunks stay on sync so the ACT stream
            # is never blocked behind a transpose.
            for sc in range(QC):
                for et in range(KT_E):
                    eng = nc.scalar if (sc == 0 and et >= 4) else nc.sync
                    eng.dma_start(
                        out=xt_c[et][sc][:],
                        in_=x_d[sc * NQ : (sc + 1) * NQ, et * P : (et + 1) * P],
                        transpose=True,
                    )
                if sc == 0:
                    for pr, w_d in ((0, wk_d), (1, wq_d)):
                        for kt in range(KT_E):
                            nc.sync.dma_start(
                                out=wkq[0][pr][kt][:],
                                in_=w_d[kt * P : (kt + 1) * P, 0:P],
                            )
            for p in range(PAIRS):
                nc.sync.dma_start(out=wo_sb[p][:],
                                  in_=wo_d[p * P : (p + 1) * P, :])
            for p in range(1, PAIRS):
                for pr, w_d in ((0, wk_d), (1, wq_d)):
                    for kt in range(KT_E):
                        nc.sync.dma_start(
                            out=wkq[p][pr][kt][:],
                            in_=w_d[kt * P : (kt + 1) * P, p * P : (p + 1) * P],
                        )

            # ---- background projection work queue -------------------------
            def v_chain(st):
                def emit():
                    vst3 = v[st].rearrange("p (h c) -> p h c", c=VW)
                    nc.vector.tensor_copy(
                        vst3[:, 0:HPC, 64:65], ones_bf[:, 0:HPC, None]
                    )
                    ps = mm_psum.tile([P, NQ], FP, tag="mm", name=f"psv{st}")
                    for kt in range(KT_E):
                        nc.tensor.matmul(
                            ps[:],
                            lhsT=xt(kt, st * P, P),
                            rhs=wv_sb[kt][:],
                            start=(kt == 0), stop=(kt == KT_E - 1),
                        )
                    nc.scalar.copy(
                        vst3[:, 0:HPC, 0:DH],
                        ps[:].rearrange("p (h c) -> p h c", c=DH),
                    )
                return emit

            def kq_chain(p, pr, nsc):
                def emit():
                    dstT = (kT, qT)[pr][p]
                    ps = mm_psum.tile([P, NQ], FP, tag="mm",
                                      name=f"pskq{p}_{pr}_{nsc}")
                    for kt in range(KT_E):
                        nc.tensor.matmul(
                            ps[:],
                            lhsT=wkq[p][pr][kt][:],
                            rhs=xt(kt, nsc * NQ, NQ),
                            start=(kt == 0), stop=(kt == KT_E - 1),
                        )
                    nc.vector.tensor_copy(dstT[:, nsc * NQ : (nsc + 1) * NQ],
                                          ps[:])
                return emit

            work = collections.deque()

            def pump(n):
                k = 0
                while work and k < n:
                    work.popleft()()
                    k += 1

            # prefill: v seq-tiles 0-3 + pair-0 k/q first 512 columns
            for st in range(QSUB):
                v_chain(st)()
            for pr in range(2):
                kq_chain(0, pr, 0)()

            # background order by deadline AND by xT seq-chunk readiness
            # (the serialized XBAR transposes finish chunk sc at ~10(sc+1)us;
            # a chain pumped before its chunk lands stalls the PE): rest of
            # pair-0 k/q and v groups interleaved by chunk, then pairs 1..3
            for g in range(1, QC):
                for pr in range(2):
                    work.append(kq_chain(0, pr, g))
                for st in range(QSUB * g, QSUB * g + QSUB):
                    work.append(v_chain(st))
            for p in range(1, PAIRS):
                for pr in range(2):
                    for nsc in range(QC):
                        work.append(kq_chain(p, pr, nsc))

            # ---- main attention loop --------------------------------------
            outproj_q = collections.deque()

            def emit_outproj(g):
                for nj in range(E // NQ):
                    po = mm_psum.tile([P, NQ], FP, tag="mm", name=f"po{g}_{nj}")
                    for pp in range(PAIRS):
                        nc.tensor.matmul(
                            po[:],
                            lhsT=attT[pp][g][:],
                            rhs=wo_sb[pp][:, nj * NQ : (nj + 1) * NQ],
                            start=(pp == 0), stop=(pp == PAIRS - 1),
                            skip_group_check=True,
                        )
                    osb = out_sb_pool.tile([P, NQ], FP, tag="osb",
                                           name=f"osb{g}_{nj}")
                    nc.vector.tensor_copy(osb[:], po[:])
                    nc.sync.dma_start(
                        out=out_d[g * P : (g + 1) * P, nj * NQ : (nj + 1) * NQ],
                        in_=osb[:],
                    )

            def finalize(p, qc, qt, g, av_t):
                # 1/sums, per-partition normalize (DVE), transpose
                rcp = rcp_pool.tile([P, 2], FP, tag="rcp", name=f"rcp{p}_{g}")
                nc.vector.reciprocal_approx_fast(
                    rcp[:, 0:1], av_t[0][:, qt * P + 64 : qt * P + 65])
                nc.vector.reciprocal_approx_fast(
                    rcp[:, 1:2], av_t[1][:, qt * P + 64 : qt * P + 65])
                att2 = att2_pool.tile([P, P], BF, tag="att2",
                                      name=f"att2_{p}_{g}")
                nc.vector.tensor_scalar_mul(
                    att2[:, 0:DH], av_t[0][:, qt * P : qt * P + DH],
                    rcp[:, 0:1])
                nc.vector.tensor_scalar_mul(
                    att2[:, DH:P], av_t[1][:, qt * P : qt * P + DH],
                    rcp[:, 1:2])
                # PE transpose (~180ns) + evac; XBAR DMA transposes measured
                # 1.2us serialized on sync and caused cross-engine convoy
                # stalls. Alternate evac engine to split the load.
                tps = mm_psum.tile([P, P], BF, tag="mm", name=f"tp{p}_{g}")
                nc.tensor.transpose(tps[:], att2[:], identity[:])
                if (p + g) % 2 == 0:
                    nc.scalar.copy(attT[p][g][:], tps[:])
                else:
                    nc.vector.tensor_copy(attT[p][g][:], tps[:])
                if p == PAIRS - 1:
                    outproj_q.append(g)

            def make_phase2_chunks(p, qc, av_t, pt_of):
                # 8 closures: per (qt, half) a SERIALIZED PSUM region group
                # (interleaved region groups within a bank corrupt on HW)
                chunks = []
                for qt in range(QSUB):
                    g = QSUB * qc + qt
                    for hf in range(2):
                        def chunk(p=p, qc=qc, qt=qt, hf=hf, g=g):
                            h = 2 * p + hf
                            for ki in range(g + 1):
                                nc.tensor.matmul(
                                    av_t[hf][:, qt * P : qt * P + VW],
                                    lhsT=pt_of[ki][:, hf * NQ + qt * P
                                                    : hf * NQ + (qt + 1) * P],
                                    rhs=v[ki][:, VW * h : VW * h + VW],
                                    start=(ki == 0), stop=(ki == g),
                                    skip_group_check=True,
                                )
                            if hf == 1:
                                finalize(p, qc, qt, g, av_t)
                        chunks.append(chunk)
                return chunks

            phase2_q = collections.deque()

            for p in range(PAIRS):
                for qc in range(QC):
                    kmax = QSUB * qc + QSUB - 1
                    # one PSUM bank per head-half; each holds the 4 q-tiles'
                    # [128, 65] accumulators at 512 B (128-col) offsets
                    av_t = [
                        av_psum.tile([P, QSUB * P], FP, tag=f"av{hf}",
                                     name=f"av{p}_{qc}_{hf}")
                        for hf in range(2)
                    ]
                    pt_of = {}
                    for ki in range(kmax + 1):
                        d = ki - QSUB * qc
                        off = P * d if d > 0 else 0
                        # ST: two K=64 head-halves
                        stp = st_psum.tile([P, 2 * NQ], FP, tag="st",
                                           name=f"st{p}_{qc}_{ki}")
                        for hf in range(2):
                            pr0 = 64 * hf
                            nc.tensor.matmul(
                                stp[:, hf * NQ + off : (hf + 1) * NQ],
                                lhsT=kT[p][pr0 : pr0 + 64,
                                           ki * P : (ki + 1) * P],
                                rhs=qT[p][pr0 : pr0 + 64,
                                          qc * NQ + off : (qc + 1) * NQ],
                                start=True, stop=True,
                            )
                        # EXP -> pt (bf16)
                        pt_t = pt_pool.tile([P, 2 * NQ], BF, tag="pt",
                                            name=f"pt{p}_{qc}_{ki}")
                        pt_of[ki] = pt_t
                        if off == 0:
                            nc.scalar.activation(
                                pt_t[:, 0 : 2 * NQ], stp[:, 0 : 2 * NQ],
                                mybir.ActivationFunctionType.Exp, scale=0.125,
                            )
                        else:
                            pt2 = pt_t.rearrange("p (k c) -> p k c", c=NQ)
                            st2 = stp.rearrange("p (k c) -> p k c", c=NQ)
                            nc.scalar.activation(
                                pt2[:, :, off:NQ], st2[:, :, off:NQ],
                                mybir.ActivationFunctionType.Exp, scale=0.125,
                            )
                        if d >= 0:
                            for hf in range(2):
                                sl = slice(hf * NQ + off, hf * NQ + off + P)
                                nc.gpsimd.tensor_tensor(
                                    pt_t[:, sl], pt_t[:, sl], ut_mask[:],
                                    mybir.AluOpType.mult,
                                )
                        pump(2 if p == 0 else 1)
                        # drain previous chunk's PV work, paced to finish
                        # within this chunk's remaining iterations
                        rem = kmax + 1 - ki
                        n = min(3, max(1, -(-len(phase2_q) // rem)))
                        for _ in range(n):
                            if phase2_q:
                                phase2_q.popleft()()
                        if p == PAIRS - 1 and len(outproj_q) > 1:
                            emit_outproj(outproj_q.popleft())
                    phase2_q.extend(make_phase2_chunks(p, qc, av_t, pt_of))

            while phase2_q:
                phase2_q.popleft()()
            pump(10**9)
            while outproj_q:
                emit_outproj(outproj_q.popleft())

    nc.compile()
    return nc


_NC_CACHE = {}


def _get_nc():
    if "nc" not in _NC_CACHE:
        _NC_CACHE["nc"] = build()
    return _NC_CACHE["nc"]


B, S, E, H, DH = 4, 2048, 1024, 16, 64
GD = (H // 2) * DH  # 512 per-core head dims


def _in_maps(x, Wq, Wk, Wv, Wo):
    import ml_dtypes

    bf = ml_dtypes.bfloat16
    maps = []
    for c in range(8):
        b, g = c // 2, c % 2
        sl = slice(g * GD, (g + 1) * GD)
        maps.append({
            "x": x[b].astype(bf),
            "wq": Wq[:, sl].astype(bf),
            "wk": Wk[:, sl].astype(bf),
            "wv": Wv[:, sl].astype(bf),
            "wo": Wo[sl, :].astype(bf),
        })
    return maps


def kernel(x, Wq, Wk, Wv, Wo):
    from concourse.bass_utils import run_bass_kernel_spmd

    x = np.asarray(x, dtype=np.float32)
    Wq = np.asarray(Wq, dtype=np.float32)
    Wk = np.asarray(Wk, dtype=np.float32)
    Wv = np.asarray(Wv, dtype=np.float32)
    Wo = np.asarray(Wo, dtype=np.float32)

    res = run_bass_kernel_spmd(
        _get_nc(), _in_maps(x, Wq, Wk, Wv, Wo), list(range(8))
    )

    out = np.empty((B, S, E), np.float32)
    for b in range(B):
        out[b] = res.results[2 * b]["out"] + res.results[2 * b + 1]["out"]
    return out
